# revision 15
# baseline (speedup 1.0000x reference)
"""ColumnParallelLinear kernel for Trainium2 (8 NeuronCores).

Computes Y[s,b,o] = sum_h X[s,b,h] * W[o,h]  (F.linear / einsum 'sbh,oh->sbo')
with S,B,H,OUT = 2048,4,1024,4096, fp32 in/out.

Strategy:
  - Flatten tokens: M = S*B = 8192 rows.  GEMM: [M,H] @ [H,OUT].
  - 2D shard over 8 cores: 4 token groups (2048 rows) x 2 out-column
    groups (2048 cols) -- minimizes per-core HBM traffic.
  - Inputs cast to bf16 on host (fp32 accumulate in PSUM keeps rel err
    ~1.5e-3, well under the 2e-2 gate).  Halves input DMA bytes and
    enables the PE's fast-weight-load path (disabled for fp32), so
    LDWEIGHTS hides fully behind the 512-cycle matmul stream.
  - Input DMA issue is split across the two HWDGE issuers (sync carries
    the w stream, scalar the x stream) so descriptor generation
    (~0.6us per dma_start) doesn't serialize the startup.
  - Chunk 0 is k-split (two single-k slices then three doubles) so the
    first matmul starts as soon as ~256KB has landed.
  - Matmuls: lhsT = x tile [128k,128m] stationary, rhs = w [128k,512n]
    moving, fp32 PSUM accumulate over KO=8 k-subtiles.
"""

import numpy as np
import ml_dtypes

import concourse.bass as bass
from concourse import bacc
import concourse.mybir as mybir
import concourse.tile as tile
from concourse.bass_utils import run_bass_kernel_spmd

S, B, H, OUT = 2048, 4, 1024, 4096
M = S * B

N_CORES = 8
G_ROW, G_COL = 4, 2          # token groups x out-feature groups
M_LOC = M // G_ROW           # 2048 rows per core
N_LOC = OUT // G_COL         # 2048 out features per core

P = 128
KO = H // P                  # 8 contraction subtiles
NT = 512                     # psum free dim (one fp32 bank)
NO = N_LOC // NT             # 4 col tiles
XG = 512                     # x chunk width (4 row tiles)
NXG = M_LOC // XG            # 4 chunks
MO = M_LOC // P              # 16 row tiles

MM_DT = mybir.dt.bfloat16

# k-split for the startup-critical chunk pair: single k-slices first so
# the PE can start after ~256KB, then doubles
SPLIT_PIECES = [(0, 1), (1, 2), (2, 4), (4, 6), (6, 8)]
WHOLE = [(0, KO)]


def build_nc(mm_dt=MM_DT):
    nc = bacc.Bacc(None, target_bir_lowering=False, enable_partition_id=False)
    # packed inputs: [chunk][partition p][k][free] so each partition's slice
    # of one chunk is contiguous in DRAM
    xH = nc.declare_dram_parameter("xH", [NXG, P, KO, XG], mm_dt,
                                   isOutput=False)
    wH = nc.declare_dram_parameter("wH", [NO, P, KO, NT], mm_dt,
                                   isOutput=False)
    y = nc.declare_dram_parameter("y", [M_LOC, N_LOC], mybir.dt.bfloat16,
                                  isOutput=True)
    y_r = y[:, :].rearrange("(mo p) n -> p mo n", p=P)

    with tile.TileContext(nc) as tc:
        with (
            tc.tile_pool(name="xp", bufs=1) as xp,
            tc.tile_pool(name="wp", bufs=1) as wp,
            tc.tile_pool(name="op", bufs=2) as op,
            tc.tile_pool(name="psp", bufs=8, space="PSUM") as psp,
        ):
            x_sb = [[] for _ in range(NXG)]
            w_sb = [[] for _ in range(NO)]

            def load_piece(pool, dram, parts, idx, k0, k1, eng, prefix):
                t = pool.tile([P, k1 - k0, dram.shape[3]], mm_dt,
                              tag=f"{prefix}{idx}k{k0}",
                              name=f"{prefix}{idx}k{k0}")
                eng.dma_start(t[:], dram[idx, :, k0:k1, :])
                parts.append((k0, k1, t))

            def slice_k(parts, k, lo, hi):
                for (k0, k1, t) in parts:
                    if k0 <= k < k1:
                        return t[:, k - k0, lo:hi]
                raise AssertionError

            # Emit input DMAs in exact consumption order.  The 16 DMA
            # engines cap at ~25GB/s each (~400GB/s/core) shared across
            # all rings, so GLOBAL arrival order must match consumption
            # order.  The k-split w0/x0 pair goes first on both issuers
            # (PE starts after the first ~256KB); every later chunk rides
            # the sync queue IN ORDER (w1..w3 for g0's n-sweep, then
            # x1..x3) -- one queue sprays across all 16 engines and gets
            # the full rate, and this stops the not-yet-needed x chunks
            # from stealing bandwidth while g0 still waits on w chunks.
            # single ordered stream on sync: the scalar-issued queue gets
            # a smaller share of the 16 shared DMA engines when both are
            # loaded, so racing x0 on scalar loses to just queueing
            # everything in consumption order on one queue (x0 first: the
            # LDWEIGHTS chain gates on it)
            # x0/w0 ride first as k-halves: one DMA engine (E79 here)
            # runs ~20% behind its peers, and a whole-chunk semaphore
            # waits for the last ring -- halving the gating pieces pulls
            # the first real matmul in by ~3us.  g0's n=0 block below
            # consumes them as two k-passes.
            load_piece(xp, xH, x_sb[0], 0, 0, 4, nc.sync, "x")
            load_piece(wp, wH, w_sb[0], 0, 0, 4, nc.sync, "w")
            load_piece(xp, xH, x_sb[0], 0, 4, 8, nc.sync, "x")
            load_piece(wp, wH, w_sb[0], 0, 4, 8, nc.sync, "w")
            for n in (1, 2, 3):
                load_piece(wp, wH, w_sb[n], n, 0, 8, nc.sync, "w")
            for g in (1, 2, 3):
                load_piece(xp, xH, x_sb[g], g, 0, 8, nc.sync, "x")

            # Warmup: dummy matmuls on a zeroed tile while the first input
            # DMA is in flight.  The PE clock-gate (HAM) needs ~3.4us of
            # sustained matmul activity to go 1.2->2.4GHz; without this the
            # first ~8us of real matmuls run at half clock.  N=256 keeps
            # the granularity fine so a ready real matmul waits <=214ns.
            wz = xp.tile([P, 256], mm_dt, tag="warm", name="warm")
            nc.vector.memset(wz[:], 0.0)
            wps = psp.tile([P, NT], mybir.dt.float32, tag="ps", name="ps")
            for _ in range(36):
                nc.tensor.matmul(wps[:, 0:256], lhsT=wz[:, 0:128],
                                 rhs=wz[:, 0:256], start=True, stop=True)

            def do_group(g, n_outer, tail=False):
                stages = [op.tile([P, N_LOC], mybir.dt.bfloat16, tag=f"st{mi}",
                                  name=f"st{g}_{mi}")
                          for mi in range(XG // P)]
                outer = range(NO) if n_outer else range(XG // P)
                inner = range(XG // P) if n_outer else range(NO)
                for a in outer:
                    if n_outer and a == 0:
                        # k-blocked first block: two passes over the
                        # x0/w0 k-halves so matmuls start on the first
                        # half without waiting for the full chunks
                        pss = [psp.tile([P, NT], mybir.dt.float32,
                                        tag="ps", name="ps")
                               for _ in inner]
                        for (k0, k1) in ((0, 4), (4, 8)):
                            for mi in inner:
                                for k in range(k0, k1):
                                    nc.tensor.matmul(
                                        pss[mi][:],
                                        lhsT=slice_k(x_sb[g], k, mi * P,
                                                     (mi + 1) * P),
                                        rhs=slice_k(w_sb[0], k, 0, NT),
                                        start=(k == 0),
                                        stop=(k == KO - 1),
                                    )
                                if k1 == KO:
                                    nc.vector.tensor_copy(
                                        stages[mi][:, 0:NT], pss[mi][:]
                                    )
                        continue
                    for b in inner:
                        n, mi = (a, b) if n_outer else (b, a)
                        ps = psp.tile([P, NT], mybir.dt.float32)
                        is_tail = tail and mi == XG // P - 1
                        if is_tail and n == NO - 1:
                            # very last tile: two independent 256-wide
                            # accumulation chains into one bank, so the
                            # first half's cast+write overlaps the second
                            # half's matmuls and the post-stream drain is
                            # one 256-wide cast + 64KB write
                            mo = g * (XG // P) + mi
                            for half in (0, 1):
                                lo, hi = half * 256, (half + 1) * 256
                                for k in range(KO):
                                    nc.tensor.matmul(
                                        ps[:, lo:hi],
                                        lhsT=slice_k(x_sb[g], k, mi * P,
                                                     (mi + 1) * P),
                                        rhs=slice_k(w_sb[n], k, lo, hi),
                                        start=(k == 0),
                                        stop=(k == KO - 1),
                                    )
                                nc.vector.tensor_copy(
                                    stages[mi][:, n * NT + lo:n * NT + hi],
                                    ps[:, lo:hi],
                                )
                                nc.sync.dma_start(
                                    y_r[:, mo, n * NT + lo:n * NT + hi],
                                    stages[mi][:, n * NT + lo:n * NT + hi],
                                )
                            continue
                        for k in range(KO):
                            nc.tensor.matmul(
                                ps[:],
                                lhsT=slice_k(x_sb[g], k, mi * P, (mi + 1) * P),
                                rhs=slice_k(w_sb[n], k, 0, NT),
                                start=(k == 0),
                                stop=(k == KO - 1),
                            )
                        nc.vector.tensor_copy(
                            stages[mi][:, n * NT:(n + 1) * NT], ps[:]
                        )
                        if is_tail:
                            # per-n writes right after each copy, on the
                            # otherwise-idle sync queue (scalar still has
                            # row writes queued at this point)
                            mo = g * (XG // P) + mi
                            nc.sync.dma_start(
                                y_r[:, mo, n * NT:(n + 1) * NT],
                                stages[mi][:, n * NT:(n + 1) * NT],
                            )
                # full-row writes; for mi-outer groups issue each row as
                # soon as its n-sweep completes so the output queue drains
                # before the end-of-kernel barrier
                last = XG // P - (1 if tail else 0)
                for mi in range(last):
                    mo = g * (XG // P) + mi
                    nc.scalar.dma_start(y_r[:, mo, :], stages[mi][:])

            do_group(0, n_outer=True)     # w arrives n-by-n
            for g in range(1, NXG):
                # mi-outer spreads the writes
                do_group(g, n_outer=False, tail=(g == NXG - 1))

    nc.compile()
    return nc


def make_in_maps(input_, weight):
    X = np.asarray(input_, dtype=np.float32).reshape(M, H)
    X = X.astype(ml_dtypes.bfloat16)
    W = np.asarray(weight, dtype=np.float32).astype(ml_dtypes.bfloat16)
    xhs = []
    for i in range(G_ROW):
        xc = X[i * M_LOC:(i + 1) * M_LOC]                 # [M_LOC, H]
        xhs.append(np.ascontiguousarray(
            xc.reshape(NXG, XG, KO, P).transpose(0, 3, 2, 1)
        ))
    whs = []
    for j in range(G_COL):
        wc = W[j * N_LOC:(j + 1) * N_LOC]                 # [N_LOC, H]
        whs.append(np.ascontiguousarray(
            wc.reshape(NO, NT, KO, P).transpose(0, 3, 2, 1)
        ))
    return [{"xH": xhs[c // G_COL], "wH": whs[c % G_COL]}
            for c in range(N_CORES)]


def assemble(results):
    Y = np.empty((M, OUT), dtype=np.float32)
    for c in range(N_CORES):
        i, j = divmod(c, G_COL)
        Y[i * M_LOC:(i + 1) * M_LOC, j * N_LOC:(j + 1) * N_LOC] = (
            results[c]["y"].astype(np.float32))
    return Y.reshape(S, B, OUT)


def kernel(input_, weight):
    nc = build_nc()
    res = run_bass_kernel_spmd(nc, make_in_maps(input_, weight), list(range(N_CORES)))
    return assemble(res.results)


# revision 18
# speedup vs baseline: 1.0030x; 1.0030x over previous
"""ColumnParallelLinear kernel for Trainium2 (8 NeuronCores).

Computes Y[s,b,o] = sum_h X[s,b,h] * W[o,h]  (F.linear / einsum 'sbh,oh->sbo')
with S,B,H,OUT = 2048,4,1024,4096, fp32 in/out.

Strategy:
  - Flatten tokens: M = S*B = 8192 rows.  GEMM: [M,H] @ [H,OUT].
  - 2D shard over 8 cores: 4 token groups (2048 rows) x 2 out-column
    groups (2048 cols) -- minimizes per-core HBM traffic.
  - Inputs cast to bf16 on host (fp32 accumulate in PSUM keeps rel err
    ~1.5e-3, well under the 2e-2 gate).  Halves input DMA bytes and
    enables the PE's fast-weight-load path (disabled for fp32), so
    LDWEIGHTS hides fully behind the 512-cycle matmul stream.
  - Input DMA issue is split across the two HWDGE issuers (sync carries
    the w stream, scalar the x stream) so descriptor generation
    (~0.6us per dma_start) doesn't serialize the startup.
  - Chunk 0 is k-split (two single-k slices then three doubles) so the
    first matmul starts as soon as ~256KB has landed.
  - Matmuls: lhsT = x tile [128k,128m] stationary, rhs = w [128k,512n]
    moving, fp32 PSUM accumulate over KO=8 k-subtiles.
"""

import numpy as np
import ml_dtypes

import concourse.bass as bass
from concourse import bacc
import concourse.mybir as mybir
import concourse.tile as tile
from concourse.bass_utils import run_bass_kernel_spmd

S, B, H, OUT = 2048, 4, 1024, 4096
M = S * B

N_CORES = 8
G_ROW, G_COL = 4, 2          # token groups x out-feature groups
M_LOC = M // G_ROW           # 2048 rows per core
N_LOC = OUT // G_COL         # 2048 out features per core

P = 128
KO = H // P                  # 8 contraction subtiles
NT = 512                     # psum free dim (one fp32 bank)
NO = N_LOC // NT             # 4 col tiles
XG = 512                     # x chunk width (4 row tiles)
NXG = M_LOC // XG            # 4 chunks
MO = M_LOC // P              # 16 row tiles

MM_DT = mybir.dt.bfloat16

# k-split for the startup-critical chunk pair: single k-slices first so
# the PE can start after ~256KB, then doubles
SPLIT_PIECES = [(0, 1), (1, 2), (2, 4), (4, 6), (6, 8)]
WHOLE = [(0, KO)]


def build_nc(mm_dt=MM_DT):
    nc = bacc.Bacc(None, target_bir_lowering=False, enable_partition_id=False)
    # packed inputs: [chunk][partition p][k][free] so each partition's slice
    # of one chunk is contiguous in DRAM
    xH = nc.declare_dram_parameter("xH", [NXG, P, KO, XG], mm_dt,
                                   isOutput=False)
    wH = nc.declare_dram_parameter("wH", [NO, P, KO, NT], mm_dt,
                                   isOutput=False)
    y = nc.declare_dram_parameter("y", [M_LOC, N_LOC], mybir.dt.bfloat16,
                                  isOutput=True)
    y_r = y[:, :].rearrange("(mo p) n -> p mo n", p=P)

    with tile.TileContext(nc) as tc:
        with (
            tc.tile_pool(name="xp", bufs=1) as xp,
            tc.tile_pool(name="wp", bufs=1) as wp,
            tc.tile_pool(name="op", bufs=2) as op,
            tc.tile_pool(name="psp", bufs=8, space="PSUM") as psp,
        ):
            x_sb = [[] for _ in range(NXG)]
            w_sb = [[] for _ in range(NO)]

            def load_piece(pool, dram, parts, idx, k0, k1, eng, prefix):
                t = pool.tile([P, k1 - k0, dram.shape[3]], mm_dt,
                              tag=f"{prefix}{idx}k{k0}",
                              name=f"{prefix}{idx}k{k0}")
                eng.dma_start(t[:], dram[idx, :, k0:k1, :])
                parts.append((k0, k1, t))

            def slice_k(parts, k, lo, hi):
                for (k0, k1, t) in parts:
                    if k0 <= k < k1:
                        return t[:, k - k0, lo:hi]
                raise AssertionError

            # Emit input DMAs in exact consumption order.  The 16 DMA
            # engines cap at ~25GB/s each (~400GB/s/core) shared across
            # all rings, so GLOBAL arrival order must match consumption
            # order.  The k-split w0/x0 pair goes first on both issuers
            # (PE starts after the first ~256KB); every later chunk rides
            # the sync queue IN ORDER (w1..w3 for g0's n-sweep, then
            # x1..x3) -- one queue sprays across all 16 engines and gets
            # the full rate, and this stops the not-yet-needed x chunks
            # from stealing bandwidth while g0 still waits on w chunks.
            # single ordered stream on sync: the scalar-issued queue gets
            # a smaller share of the 16 shared DMA engines when both are
            # loaded, so racing x0 on scalar loses to just queueing
            # everything in consumption order on one queue (x0 first: the
            # LDWEIGHTS chain gates on it)
            # x0/w0 ride first as k-halves: one DMA engine (E79 here)
            # runs ~20% behind its peers, and a whole-chunk semaphore
            # waits for the last ring -- halving the gating pieces pulls
            # the first real matmul in by ~3us.  g0's n=0 block below
            # consumes them as two k-passes.
            load_piece(xp, xH, x_sb[0], 0, 0, 4, nc.sync, "x")
            load_piece(wp, wH, w_sb[0], 0, 0, 4, nc.sync, "w")
            load_piece(xp, xH, x_sb[0], 0, 4, 8, nc.sync, "x")
            load_piece(wp, wH, w_sb[0], 0, 4, 8, nc.sync, "w")
            for n in (1, 2, 3):
                load_piece(wp, wH, w_sb[n], n, 0, 8, nc.sync, "w")
            for g in (1, 2, 3):
                load_piece(xp, xH, x_sb[g], g, 0, 8, nc.sync, "x")

            # Warmup: dummy matmuls on a zeroed tile while the first input
            # DMA is in flight.  The PE clock-gate (HAM) needs ~3.4us of
            # sustained matmul activity to go 1.2->2.4GHz; without this the
            # first ~8us of real matmuls run at half clock.  N=256 keeps
            # the granularity fine so a ready real matmul waits <=214ns.
            wz = xp.tile([P, 256], mm_dt, tag="warm", name="warm")
            nc.vector.memset(wz[:], 0.0)
            wps = psp.tile([P, NT], mybir.dt.float32, tag="ps", name="ps")
            for _ in range(36):
                nc.tensor.matmul(wps[:, 0:256], lhsT=wz[:, 0:128],
                                 rhs=wz[:, 0:256], start=True, stop=True)

            def do_group(g, n_outer, tail=False):
                stages = [op.tile([P, N_LOC], mybir.dt.bfloat16, tag=f"st{mi}",
                                  name=f"st{g}_{mi}")
                          for mi in range(XG // P)]
                outer = range(NO) if n_outer else range(XG // P)
                inner = range(XG // P) if n_outer else range(NO)
                for a in outer:
                    if n_outer and a == 0:
                        # k-blocked first block: two passes over the
                        # x0/w0 k-halves so matmuls start on the first
                        # half without waiting for the full chunks
                        pss = [psp.tile([P, NT], mybir.dt.float32,
                                        tag="ps", name="ps")
                               for _ in inner]
                        for (k0, k1) in ((0, 4), (4, 8)):
                            for mi in inner:
                                for k in range(k0, k1):
                                    nc.tensor.matmul(
                                        pss[mi][:],
                                        lhsT=slice_k(x_sb[g], k, mi * P,
                                                     (mi + 1) * P),
                                        rhs=slice_k(w_sb[0], k, 0, NT),
                                        start=(k == 0),
                                        stop=(k == KO - 1),
                                    )
                                if k1 == KO:
                                    nc.vector.tensor_copy(
                                        stages[mi][:, 0:NT], pss[mi][:]
                                    )
                        continue
                    for b in inner:
                        n, mi = (a, b) if n_outer else (b, a)
                        ps = psp.tile([P, NT], mybir.dt.float32)
                        is_tail = tail
                        if is_tail and n == NO - 1 and mi == XG // P - 1:
                            # very last tile: two independent 256-wide
                            # accumulation chains into one bank, so the
                            # first half's cast+write overlaps the second
                            # half's matmuls and the post-stream drain is
                            # one 256-wide cast + 64KB write
                            mo = g * (XG // P) + mi
                            for half in (0, 1):
                                lo, hi = half * 256, (half + 1) * 256
                                for k in range(KO):
                                    nc.tensor.matmul(
                                        ps[:, lo:hi],
                                        lhsT=slice_k(x_sb[g], k, mi * P,
                                                     (mi + 1) * P),
                                        rhs=slice_k(w_sb[n], k, lo, hi),
                                        start=(k == 0),
                                        stop=(k == KO - 1),
                                    )
                                nc.vector.tensor_copy(
                                    stages[mi][:, n * NT + lo:n * NT + hi],
                                    ps[:, lo:hi],
                                )
                                nc.sync.dma_start(
                                    y_r[:, mo, n * NT + lo:n * NT + hi],
                                    stages[mi][:, n * NT + lo:n * NT + hi],
                                )
                            continue
                        for k in range(KO):
                            nc.tensor.matmul(
                                ps[:],
                                lhsT=slice_k(x_sb[g], k, mi * P, (mi + 1) * P),
                                rhs=slice_k(w_sb[n], k, 0, NT),
                                start=(k == 0),
                                stop=(k == KO - 1),
                            )
                        nc.vector.tensor_copy(
                            stages[mi][:, n * NT:(n + 1) * NT], ps[:]
                        )
                        if is_tail:
                            # the whole tail group writes per-n pieces right
                            # after each copy (alternating issuers) so no
                            # row-write backlog drains after the last matmul
                            mo = g * (XG // P) + mi
                            eng = nc.sync if mi == XG // P - 1 else nc.scalar
                            eng.dma_start(
                                y_r[:, mo, n * NT:(n + 1) * NT],
                                stages[mi][:, n * NT:(n + 1) * NT],
                            )
                # full-row writes; for mi-outer groups issue each row as
                # soon as its n-sweep completes so the output queue drains
                # before the end-of-kernel barrier
                last = XG // P - (1 if tail else 0)
                for mi in range(last):
                    mo = g * (XG // P) + mi
                    nc.scalar.dma_start(y_r[:, mo, :], stages[mi][:])

            do_group(0, n_outer=True)     # w arrives n-by-n
            for g in range(1, NXG):
                # mi-outer spreads the writes
                do_group(g, n_outer=False, tail=(g == NXG - 1))

    nc.compile()
    return nc


def make_in_maps(input_, weight):
    X = np.asarray(input_, dtype=np.float32).reshape(M, H)
    X = X.astype(ml_dtypes.bfloat16)
    W = np.asarray(weight, dtype=np.float32).astype(ml_dtypes.bfloat16)
    xhs = []
    for i in range(G_ROW):
        xc = X[i * M_LOC:(i + 1) * M_LOC]                 # [M_LOC, H]
        xhs.append(np.ascontiguousarray(
            xc.reshape(NXG, XG, KO, P).transpose(0, 3, 2, 1)
        ))
    whs = []
    for j in range(G_COL):
        wc = W[j * N_LOC:(j + 1) * N_LOC]                 # [N_LOC, H]
        whs.append(np.ascontiguousarray(
            wc.reshape(NO, NT, KO, P).transpose(0, 3, 2, 1)
        ))
    return [{"xH": xhs[c // G_COL], "wH": whs[c % G_COL]}
            for c in range(N_CORES)]


def assemble(results):
    Y = np.empty((M, OUT), dtype=np.float32)
    for c in range(N_CORES):
        i, j = divmod(c, G_COL)
        Y[i * M_LOC:(i + 1) * M_LOC, j * N_LOC:(j + 1) * N_LOC] = (
            results[c]["y"].astype(np.float32))
    return Y.reshape(S, B, OUT)


def kernel(input_, weight):
    nc = build_nc()
    res = run_bass_kernel_spmd(nc, make_in_maps(input_, weight), list(range(N_CORES)))
    return assemble(res.results)
